# revision 1
# baseline (speedup 1.0000x reference)
"""Trainium2 Bass kernel for the EdgeMask problem.

Computes, for h (B,T,N,d), I_full (B,T,N,N), MLP params W1 (2d,hid) b1 (hid,)
W2 (hid,) b2 (1,):
    li = h @ W1[:d]; lj = h @ W1[d:]
    hid = relu(li[:,:,:,None,:] + lj[:,:,None,:,:] + b1)
    M = sigmoid(hid @ W2 + b2);  I_sparse = I_full * M
Returns (I_sparse, M).

Sharding: data-parallel over B across 8 NeuronCores (B=8), no collectives.

Per-core layout (per (t) slice, N=128 nodes, d=128, K=32 hidden):
  - hT = h[t].T via PE transpose (d on partitions)
  - liT/ljT via col-tiled PE matmuls with W1a/W1b as stationary operands.
    Partition stacking p = 32*gp + k (4 replicas of the 32 hidden units).
    "Group" g covers rows i in {g, g+32, g+64, g+96} (i = g + 32*gp).
      R[32gp+k, j]  = lj[j,k] + b1[k]        (replicated 4x, ACT adds b1)
      S[32gp+k, g]  = li[g+32gp, k]          (li "stack", fp32)
  - Pointwise (the N^2*K hot loop): for each group g one fused op
      hid_g = max(R + S[:,g], 0)   -- DVE tensor_scalar(add,max) / ACT Relu+bias
  - Reduce over k via PE: lhsT = blockdiag(W2 x4) (128,4), col-tiled 4-way,
    rhs = 4 groups' hid packed into (128,512):
      logits'[i=4w+c+32m, j] at PSUM[32q+m, 128c+j]  (w = 4*half + q)
  - Compact 2 PSUM banks -> dense (128,128) via DMA, sigmoid(+b2) on ACT,
    I_full * M on DVE, DMA out.
"""

import functools

import numpy as np

import concourse.bass as bass
import concourse.mybir as mybir
import concourse.tile as tile
from concourse import bacc

F32 = mybir.dt.float32
F16 = mybir.dt.float16

B = 8
T = 32
N = 128
D = 128
K = 32  # hidden
NCORES = 8

AFT = mybir.ActivationFunctionType
ALU = mybir.AluOpType

# dtype of the hid (pointwise+reduce) path: F16 -> DVE 4x mode, F32 exact
HID_DT = F16
HID_NP = np.float16 if HID_DT == F16 else np.float32

# pointwise split: first ACT_SHARE_G groups on ACT, last POOL_SHARE_G on
# GPSIMD, rest on DVE
ACT_SHARE_G = 3
POOL_SHARE_G = 10
HID_BUFS = 24
SIG_DENSE = False
PW_SPREAD = False
MASK_ON_POOL = False
IO_BUFS = 4
OUT_BUFS = 3
RS_BUFS = 3
LILJ_BUFS = 2
MP_BUFS = 2
S_ON_ACT = False
MEXIT_SPLIT = False


def _pw_engine(g):
    if PW_SPREAD:
        # interleave: pool every 3rd, act sprinkled, rest dve
        w, c = divmod(g, 4)
        if c == 3 and w >= 8 - POOL_SHARE_G // 4 * 4:
            pass
        seq = (["dve"] * (K - ACT_SHARE_G - POOL_SHARE_G)
               + ["pool"] * POOL_SHARE_G + ["act"] * ACT_SHARE_G)
        # round-robin-ish deterministic shuffle
        return seq[(g * 7) % K]
    if g < ACT_SHARE_G:
        return "act"
    if g >= K - POOL_SHARE_G:
        return "pool"
    return "dve"


def _build(t_slices: int = T, skip=()):
    nc = bacc.Bacc(
        "TRN2", target_bir_lowering=False, debug=False, num_devices=NCORES
    )

    ht_d = nc.dram_tensor("ht", [D, t_slices * N], HID_DT, kind="ExternalInput")
    i_d = nc.dram_tensor("ifull", [t_slices, N, N], F32, kind="ExternalInput")
    w1a_d = nc.dram_tensor("w1a", [D, K], HID_DT, kind="ExternalInput")
    w1b_d = nc.dram_tensor("w1b", [D, K], HID_DT, kind="ExternalInput")
    b1t_d = nc.dram_tensor("b1t", [128, 1], F32, kind="ExternalInput")
    wd_d = nc.dram_tensor("wd", [128, 32], HID_DT, kind="ExternalInput")
    b2t_d = nc.dram_tensor("b2t", [128, 1], F32, kind="ExternalInput")
    perm_d = nc.dram_tensor("perm", [128, 8 * 128], HID_DT, kind="ExternalInput")

    # merged output: [..., 0:N] = M, [..., N:2N] = I_sparse (one store per slice)
    mi_d = nc.dram_tensor("mi", [t_slices, N, 2 * N], F32, kind="ExternalOutput")

    with tile.TileContext(nc) as tc:
        with (
            tc.tile_pool(name="const", bufs=1) as cpool,
            tc.tile_pool(name="hin", bufs=4) as hpool,
            tc.tile_pool(name="hts", bufs=3) as htpool,
            tc.tile_pool(name="rs", bufs=RS_BUFS) as rspool,
            tc.tile_pool(name="hid", bufs=HID_BUFS) as hidpool,
            tc.tile_pool(name="io", bufs=IO_BUFS) as iopool,
            tc.tile_pool(name="outp", bufs=OUT_BUFS) as opool,
            tc.tile_pool(name="psum", bufs=2, space="PSUM") as ppool,
        ):
            w1a_sb = cpool.tile([D, K], HID_DT)
            nc.sync.dma_start(w1a_sb[:], w1a_d[:])
            w1b_sb = cpool.tile([D, K], HID_DT)
            nc.sync.dma_start(w1b_sb[:], w1b_d[:])
            b1t_sb = cpool.tile([128, 1], F32)
            nc.sync.dma_start(b1t_sb[:], b1t_d[:])
            wd_sb = cpool.tile([128, 32], HID_DT)
            nc.sync.dma_start(wd_sb[:], wd_d[:])
            b2t_sb = cpool.tile([128, 1], F32)
            nc.sync.dma_start(b2t_sb[:], b2t_d[:])
            perm_sb = cpool.tile([128, 8 * 128], HID_DT)
            nc.sync.dma_start(perm_sb[:], perm_d[:])
            # all slices' hT, chunked so slice 0 can start early
            htall_sb = cpool.tile([D, t_slices * N], HID_DT)
            n_chunks = min(8, t_slices)
            chunk = t_slices * N // n_chunks
            for ci in range(n_chunks):
                nc.sync.dma_start(
                    htall_sb[:, ci * chunk : (ci + 1) * chunk],
                    ht_d[:, ci * chunk : (ci + 1) * chunk],
                )

            for t in range(t_slices):
                ht_sb = htall_sb[:, t * N : (t + 1) * N]

                # ---- liT / ljT, col-tiled (4 concurrent 32-col groups) ----
                lilj_ps = ppool.tile([128, N + K], F32, tag="lilj", bufs=LILJ_BUFS)
                for gp in range(4):
                    # ljT replicated: out[32gp+k, j] = lj[j, k]
                    nc.tensor.matmul(
                        lilj_ps[32 * gp : 32 * gp + 32, 0:N],
                        w1b_sb[:],
                        ht_sb,
                        tile_position=(0, 32 * gp),
                        skip_group_check=True,
                    )
                for gp in range(4):
                    # li stack: out[32gp+k, g] = li[g+32gp, k]
                    nc.tensor.matmul(
                        lilj_ps[32 * gp : 32 * gp + 32, N : N + K],
                        w1a_sb[:],
                        ht_sb[:, 32 * gp : 32 * gp + 32],
                        tile_position=(0, 32 * gp),
                        skip_group_check=True,
                    )

                # R = ljT_rep + b1 (cast to HID_DT); S = li stack (fp32)
                r_sb = rspool.tile([128, N], HID_DT, tag="r")
                nc.scalar.activation(
                    r_sb[:], lilj_ps[:, 0:N], AFT.Identity, bias=b1t_sb[:, 0:1]
                )
                s_sb = rspool.tile([128, K], F32, tag="s")
                if S_ON_ACT:
                    nc.scalar.copy(s_sb[:], lilj_ps[:, N : N + K])
                else:
                    nc.vector.tensor_copy(s_sb[:], lilj_ps[:, N : N + K])

                # ---- pointwise: hid_g = relu(R + S[:, g]) ----
                hbufs = [
                    hidpool.tile([128, 4 * N], HID_DT, tag="hid", name=f"hb{w}")
                    for w in range(8)
                ]
                for g in range(K):
                    w, c = divmod(g, 4)
                    dst = hbufs[w][:, c * N : (c + 1) * N]
                    eng = _pw_engine(g)
                    if eng == "act":
                        nc.scalar.activation(
                            dst, r_sb[:], AFT.Relu, bias=s_sb[:, g : g + 1]
                        )
                    elif eng == "pool":
                        nc.gpsimd.tensor_scalar(
                            dst, r_sb[:], s_sb[:, g : g + 1], 0.0, ALU.add, ALU.max
                        )
                    else:
                        nc.vector.tensor_scalar(
                            dst, r_sb[:], s_sb[:, g : g + 1], 0.0, ALU.add, ALU.max
                        )

                # ---- reduce over k on PE (col-tiled, 2 waves of 4) ----
                l_ps = [
                    ppool.tile([128, 4 * N], F32, tag="l0", name="l0"),
                    ppool.tile([128, 4 * N], F32, tag="l1", name="l1"),
                ]
                for w in range(8):
                    half, q = divmod(w, 4)
                    nc.tensor.matmul(
                        l_ps[half][32 * q : 32 * q + 32, :],
                        wd_sb[:],
                        hbufs[w][:],
                        tile_position=(0, 32 * q),
                    )

                # ---- PSUM exits (cast fp16) to SBUF ----
                # SIG_DENSE: raw-logit copies here, sigmoid after the permute.
                # else: sigmoid(+b2) applied here (sparse), permute carries M.
                lsp = [
                    opool.tile([128, 4 * N], HID_DT, tag="lsp0", name="lsp0"),
                    opool.tile([128, 4 * N], HID_DT, tag="lsp1", name="lsp1"),
                ]
                if SIG_DENSE:
                    nc.scalar.copy(lsp[0][:], l_ps[0][:])
                    nc.vector.tensor_copy(lsp[1][:], l_ps[1][:])
                else:
                    for half in range(2):
                        nc.scalar.activation(
                            lsp[half][:], l_ps[half][:], AFT.Sigmoid,
                            bias=b2t_sb[:, 0:1],
                        )

                # ---- un-permute logits on PE: 8 accumulating matmuls with 0/1
                # permutation matrices; P_b[32q+m, 32m+16h+4q+c] = 1, b=4h+c
                mp_ps = ppool.tile([128, N], F32, tag="mp", bufs=MP_BUFS)
                for b in range(8):
                    h, c = divmod(b, 4)
                    nc.tensor.matmul(
                        mp_ps[:],
                        perm_sb[:, 128 * b : 128 * (b + 1)],
                        lsp[h][:, 128 * c : 128 * (c + 1)],
                        start=(b == 0),
                        stop=(b == 7),
                    )
                # dense-PSUM exit; M lands in mi[:, 0:N]
                mi_sb = opool.tile([128, 2 * N], F32, tag="mi")
                if SIG_DENSE:
                    nc.scalar.activation(
                        mi_sb[:, 0:N], mp_ps[:], AFT.Sigmoid, bias=b2t_sb[:, 0:1]
                    )
                elif MEXIT_SPLIT:
                    nc.vector.tensor_copy(mi_sb[:, 0 : N // 2], mp_ps[:, 0 : N // 2])
                    nc.scalar.copy(mi_sb[:, N // 2 : N], mp_ps[:, N // 2 : N])
                else:
                    nc.vector.tensor_copy(mi_sb[:, 0:N], mp_ps[:])
                i_sb = iopool.tile([N, N], F32, tag="i")
                nc.sync.dma_start(i_sb[:], i_d[t, :, :])
                if MASK_ON_POOL:
                    nc.gpsimd.tensor_tensor(
                        mi_sb[:, N : 2 * N], i_sb[:], mi_sb[:, 0:N], ALU.mult
                    )
                else:
                    nc.vector.tensor_tensor(
                        mi_sb[:, N : 2 * N], i_sb[:], mi_sb[:, 0:N], ALU.mult
                    )
                nc.sync.dma_start(mi_d[t, :, :], mi_sb[:])

    nc.compile()
    return nc


def make_aux_inputs(W1, b1, W2, b2):
    W1 = np.asarray(W1)
    w1a = np.ascontiguousarray(W1[:D]).astype(HID_NP)
    w1b = np.ascontiguousarray(W1[D:]).astype(HID_NP)
    b1t = np.ascontiguousarray(np.tile(np.asarray(b1, np.float32), 4).reshape(128, 1))
    # col m carries W2 at partition-block (m % 4): every PSUM output row of the
    # reduce matmul is then a valid (replicated) logits row
    wd = np.zeros((128, 32), HID_NP)
    for m in range(32):
        gp = m % 4
        wd[32 * gp : 32 * gp + 32, m] = np.asarray(W2)
    b2t = np.full((128, 1), np.asarray(b2, np.float32)[0], np.float32)
    perm = np.zeros((8, 128, 128), np.float32)
    for h in range(2):
        for c in range(4):
            for q in range(4):
                for m in range(4):
                    perm[4 * h + c, 32 * q + m, 32 * m + 16 * h + 4 * q + c] = 1.0
    perm = np.ascontiguousarray(np.concatenate(list(perm), axis=1)).astype(HID_NP)
    return {
        "perm": perm,
        "w1a": w1a,
        "w1b": w1b,
        "b1t": b1t,
        "wd": wd,
        "b2t": b2t,
    }


TRACE = False
LAST_RESULTS = None


@functools.lru_cache(maxsize=1)
def _built_nc():
    return _build(T)


def kernel(**inputs):
    from concourse.bass_utils import run_bass_kernel_spmd

    h = np.asarray(inputs["h"])
    # (B, T, N, D) -> (B, D, T*N) so one DMA per core loads all hT with 8KB runs
    ht = np.ascontiguousarray(np.transpose(h, (0, 3, 1, 2)).reshape(B, D, -1)).astype(
        HID_NP
    )
    ifull = np.ascontiguousarray(np.asarray(inputs["I_full"], np.float32))
    aux = make_aux_inputs(inputs["W1"], inputs["b1"], inputs["W2"], inputs["b2"])

    nc = _built_nc()
    in_maps = [{"ht": ht[c], "ifull": ifull[c], **aux} for c in range(NCORES)]
    res = run_bass_kernel_spmd(
        nc, in_maps, core_ids=list(range(NCORES)), trace=TRACE
    )
    global LAST_RESULTS
    LAST_RESULTS = res
    mi = np.stack([res.results[c]["mi"] for c in range(NCORES)])
    return np.ascontiguousarray(mi[..., N:]), np.ascontiguousarray(mi[..., :N])



# revision 14
# speedup vs baseline: 1.2552x; 1.2552x over previous
"""Trainium2 Bass kernel for the EdgeMask problem.

Computes, for h (B,T,N,d), I_full (B,T,N,N), MLP params W1 (2d,hid) b1 (hid,)
W2 (hid,) b2 (1,):
    li = h @ W1[:d]; lj = h @ W1[d:]
    hid = relu(li[:,:,:,None,:] + lj[:,:,None,:,:] + b1)
    M = sigmoid(hid @ W2 + b2);  I_sparse = I_full * M
Returns (I_sparse, M).

Sharding: data-parallel over B across 8 NeuronCores (B=8), no collectives.

Per-core layout (per t slice, N=128 nodes, d=128, K=32 hidden), processed in
slice PAIRS so fixed per-slice ops amortize:
  - hT = h[t].T (d on partitions), fp16.
  - R = ljT replicated 4x via ONE matmul with a column-replicated lhsT
    (w1brep[d, 32gp+k] = W1b[d,k]); S = li stack via 4 col-block matmuls.
    Partition stacking p = 32*gp + k. Both slices of a pair share one PSUM
    tile; one 256-col R exit (pure copy) + one 64-col S exit (+b1).
  - Pointwise hot loop (N^2*K): group g covers rows i in {g+32gp}:
      hid_g[32gp+k, j] = relu(R[:,j] + S[:,g])   (tensor_scalar add,max)
    split across DVE / GPSIMD / ACT.
  - Reduce over k on PE: per g (= 8q+m) one matmul with a 32-col lhsT
    that is zero except a 4-col stripe (ws32[32gp+k, 32m+4m'+gp] = W2[k]
    at stripe m'=m), 8 matmuls accumulating into PSUM block [32q, 32q+32):
      mp[32q+4m+gp, j] = logits[g+32gp, j]   -- ROW-PERMUTED dense logits.
    Rows are unpermuted on the host (I_full pre-permuted to match).
  - Two slices' logits share one PSUM tile -> one 256-col sigmoid on ACT,
    one 256-col mask multiply on DVE. M and I_sparse go to separate DRAM
    tensors, stored with 4-slice-batched DMAs.
"""

import functools

import numpy as np

import concourse.bass as bass
import concourse.mybir as mybir
import concourse.tile as tile
from concourse import bacc

F32 = mybir.dt.float32
F16 = mybir.dt.float16

B = 8
T = 32
N = 128
D = 128
K = 32  # hidden
NCORES = 8

AFT = mybir.ActivationFunctionType
ALU = mybir.AluOpType

HID_DT = F16
HID_NP = np.float16

# pointwise engine split: counts per 32 groups
ACT_G = 5
POOL_G = 7
# fixed-op engine assignment
R_ENG = "act"    # paired R exit: pure copy PSUM->SBUF fp16, 256 cols (GPSIMD cannot touch PSUM)
S_ENG = "dve"    # paired S exit: +b1, 64 cols, fp32 (GPSIMD cannot touch PSUM)
MASK_ENG = "dve"
IO_BATCH = 4
HID_BUFS = 24
RS_BUFS = 3
IO_BUFS = 3
OUT_BUFS = 3
LILJ_BUFS = 2
MP_BUFS = 2


def _pw_seq():
    """Interleaved engine sequence for the 32 pointwise groups."""
    counts = {"act": ACT_G, "pool": POOL_G, "dve": K - ACT_G - POOL_G}
    seq = []
    acc = {e: 0.0 for e in counts}
    for _ in range(K):
        for e in counts:
            acc[e] += counts[e] / K
        pick = max(acc, key=lambda e: acc[e])
        acc[pick] -= 1.0
        seq.append(pick)
    return seq


def _eng_ts(nc, eng, dst, src, scalar1, scalar2, op0, op1=None):
    ns = {"pool": nc.gpsimd, "act": nc.scalar, "dve": nc.vector}[eng]
    if op1 is None:
        ns.tensor_scalar(dst, src, scalar1, scalar2, op0)
    else:
        ns.tensor_scalar(dst, src, scalar1, scalar2, op0, op1)


def _eng_copy(nc, eng, dst, src):
    if eng == "pool":
        nc.gpsimd.tensor_copy(dst, src)
    elif eng == "act":
        nc.scalar.copy(dst, src)
    else:
        nc.vector.tensor_copy(dst, src)


def _build(t_slices: int = T):
    nc = bacc.Bacc(
        "TRN2", target_bir_lowering=False, debug=False, num_devices=NCORES
    )

    ht_d = nc.dram_tensor("ht", [D, t_slices * N], HID_DT, kind="ExternalInput")
    i_d = nc.dram_tensor("ifull", [N, t_slices * N], HID_DT, kind="ExternalInput")
    w1a_d = nc.dram_tensor("w1a", [D, K], HID_DT, kind="ExternalInput")
    w1brep_d = nc.dram_tensor("w1brep", [D, 128], HID_DT, kind="ExternalInput")
    b1t_d = nc.dram_tensor("b1t", [128, 1], F32, kind="ExternalInput")
    ws32_d = nc.dram_tensor("ws32", [128, 8 * 32], HID_DT, kind="ExternalInput")
    b2t_d = nc.dram_tensor("b2t", [128, 1], F32, kind="ExternalInput")

    # row-permuted outputs
    m_d = nc.dram_tensor("m", [N, t_slices * N], HID_DT, kind="ExternalOutput")
    isp_d = nc.dram_tensor("isp", [N, t_slices * N], HID_DT, kind="ExternalOutput")

    pw_seq = _pw_seq()

    with tile.TileContext(nc) as tc:
        with (
            tc.tile_pool(name="const", bufs=1) as cpool,
            tc.tile_pool(name="rs", bufs=RS_BUFS) as rspool,
            tc.tile_pool(name="hid", bufs=HID_BUFS) as hidpool,
            tc.tile_pool(name="io", bufs=IO_BUFS) as iopool,
            tc.tile_pool(name="outp", bufs=OUT_BUFS) as opool,
            tc.tile_pool(name="psum", bufs=2, space="PSUM") as ppool,
        ):
            # consts needed first for slice 0, in dependency order
            w1a_sb = cpool.tile([D, K], HID_DT)
            nc.sync.dma_start(w1a_sb[:], w1a_d[:])
            w1brep_sb = cpool.tile([D, 128], HID_DT)
            nc.sync.dma_start(w1brep_sb[:], w1brep_d[:])
            b1t_sb = cpool.tile([128, 1], F32)
            nc.sync.dma_start(b1t_sb[:], b1t_d[:])
            htall_sb = cpool.tile([D, t_slices * N], HID_DT)
            n_chunks = min(8, t_slices)
            chunk = t_slices * N // n_chunks
            nc.sync.dma_start(htall_sb[:, 0:chunk], ht_d[:, 0:chunk])
            ws32_sb = cpool.tile([128, 8 * 32], HID_DT)
            nc.sync.dma_start(ws32_sb[:], ws32_d[:])
            b2t_sb = cpool.tile([128, 1], F32)
            nc.sync.dma_start(b2t_sb[:], b2t_d[:])

            i4_cur = None
            i4_next = iopool.tile([128, IO_BATCH * N], HID_DT, tag="i4")
            nc.sync.dma_start(i4_next[:], i_d[:, 0 : IO_BATCH * N])
            m4_sb = None
            isp4_sb = None

            # remaining hT chunks
            for ci in range(1, n_chunks):
                nc.sync.dma_start(
                    htall_sb[:, ci * chunk : (ci + 1) * chunk],
                    ht_d[:, ci * chunk : (ci + 1) * chunk],
                )

            for t0 in range(0, t_slices, 2):
                slot = t0 % IO_BATCH
                if slot == 0:
                    i4_cur = i4_next
                    if t0 + IO_BATCH < t_slices:
                        i4_next = iopool.tile([128, IO_BATCH * N], HID_DT, tag="i4")
                        nc.sync.dma_start(
                            i4_next[:],
                            i_d[:, (t0 + IO_BATCH) * N : (t0 + 2 * IO_BATCH) * N],
                        )
                    m4_sb = opool.tile([128, IO_BATCH * N], HID_DT, tag="m4")
                    isp4_sb = opool.tile([128, IO_BATCH * N], HID_DT, tag="isp4")

                # ---- lilj for both slices of the pair into one PSUM tile ----
                # layout: [R(t0) | R(t0+1) | S(t0) | S(t0+1)]
                lilj_ps = ppool.tile(
                    [128, 2 * N + 2 * K], F32, tag="lilj", bufs=LILJ_BUFS
                )
                for h in range(2):
                    ht_sb = htall_sb[:, (t0 + h) * N : (t0 + h + 1) * N]
                    nc.tensor.matmul(
                        lilj_ps[:, h * N : (h + 1) * N],
                        w1brep_sb[:],
                        ht_sb,
                        skip_group_check=True,
                    )
                    for gp in range(4):
                        nc.tensor.matmul(
                            lilj_ps[
                                32 * gp : 32 * gp + 32,
                                2 * N + h * K : 2 * N + (h + 1) * K,
                            ],
                            w1a_sb[:],
                            ht_sb[:, 32 * gp : 32 * gp + 32],
                            tile_position=(0, 32 * gp),
                            skip_group_check=True,
                        )

                # paired exits: R (256 cols, pure copy), S (64 cols, +b1)
                r2_sb = rspool.tile([128, 2 * N], HID_DT, tag="r2")
                _eng_copy(nc, R_ENG, r2_sb[:], lilj_ps[:, 0 : 2 * N])
                s2_sb = rspool.tile([128, 2 * K], F32, tag="s2")
                if S_ENG == "act":
                    nc.scalar.activation(
                        s2_sb[:], lilj_ps[:, 2 * N : 2 * N + 2 * K],
                        AFT.Identity, bias=b1t_sb[:, 0:1],
                    )
                else:
                    _eng_ts(
                        nc, S_ENG, s2_sb[:], lilj_ps[:, 2 * N : 2 * N + 2 * K],
                        b1t_sb[:, 0:1], None, ALU.add,
                    )

                # per-pair PSUM logits tile
                mp2_ps = ppool.tile([128, 2 * N], F32, tag="mp", bufs=MP_BUFS)

                for h in range(2):
                    r_sb = r2_sb[:, h * N : (h + 1) * N]
                    s_sb = s2_sb[:, h * K : (h + 1) * K]

                    # ---- pointwise: hid_g = relu(R + S[:, g]) ----
                    hbufs = [
                        hidpool.tile([128, 4 * N], HID_DT, tag="hid", name=f"hb{w}")
                        for w in range(8)
                    ]
                    for g in range(K):
                        w, c = divmod(g, 4)
                        dst = hbufs[w][:, c * N : (c + 1) * N]
                        eng = pw_seq[g]
                        if eng == "act":
                            nc.scalar.activation(
                                dst, r_sb, AFT.Relu, bias=s_sb[:, g : g + 1]
                            )
                        else:
                            _eng_ts(
                                nc, eng, dst, r_sb, s_sb[:, g : g + 1], 0.0,
                                ALU.add, ALU.max,
                            )

                    # ---- reduce over k on PE: row-permuted dense logits ----
                    # g = 8q+m: mp[32q+4m+gp, h*N+j] = logits[g+32gp, j]
                    for g in range(K):
                        w, c = divmod(g, 4)
                        q, m = divmod(g, 8)
                        nc.tensor.matmul(
                            mp2_ps[32 * q : 32 * q + 32, h * N : (h + 1) * N],
                            ws32_sb[:, 32 * m : 32 * m + 32],
                            hbufs[w][:, c * N : (c + 1) * N],
                            start=(m == 0),
                            stop=(m == 7),
                            tile_position=(0, 32 * q),
                            skip_group_check=True,
                        )

                # ---- per pair: sigmoid(+b2) then I_perm * M ----
                pcol = slot * N
                nc.scalar.activation(
                    m4_sb[:, pcol : pcol + 2 * N],
                    mp2_ps[:],
                    AFT.Sigmoid,
                    bias=b2t_sb[:, 0:1],
                )
                if MASK_ENG == "pool":
                    nc.gpsimd.tensor_tensor(
                        isp4_sb[:, pcol : pcol + 2 * N],
                        i4_cur[:, pcol : pcol + 2 * N],
                        m4_sb[:, pcol : pcol + 2 * N],
                        ALU.mult,
                    )
                else:
                    nc.vector.tensor_tensor(
                        isp4_sb[:, pcol : pcol + 2 * N],
                        i4_cur[:, pcol : pcol + 2 * N],
                        m4_sb[:, pcol : pcol + 2 * N],
                        ALU.mult,
                    )
                if slot + 2 == IO_BATCH:
                    base = (t0 + 2 - IO_BATCH) * N
                    nc.sync.dma_start(
                        m_d[:, base : base + IO_BATCH * N], m4_sb[:]
                    )
                    nc.sync.dma_start(
                        isp_d[:, base : base + IO_BATCH * N], isp4_sb[:]
                    )

    nc.compile()
    return nc


# permutation: PSUM/DRAM row p' = 32q+4m+gp holds logits row i = (8q+m) + 32*gp
PERM = np.array(
    [8 * (p // 32) + ((p % 32) // 4) + 32 * (p % 4) for p in range(128)],
    dtype=np.int64,
)


def make_aux_inputs(W1, b1, W2, b2):
    W1 = np.asarray(W1)
    w1a = np.ascontiguousarray(W1[:D]).astype(HID_NP)
    w1b = np.ascontiguousarray(W1[D:]).astype(HID_NP)  # (D, K)
    # w1brep[d, 32gp+k] = W1b[d, k]
    w1brep = np.ascontiguousarray(np.tile(w1b, (1, 4))).astype(HID_NP)
    b1t = np.ascontiguousarray(
        np.tile(np.asarray(b1, np.float32), 4).reshape(128, 1)
    )
    # ws32[:, 32m:32m+32][32gp+k, 4m+gp] = W2[k]; zero elsewhere
    ws32 = np.zeros((128, 8, 32), HID_NP)
    for m in range(8):
        for gp in range(4):
            ws32[32 * gp : 32 * gp + 32, m, 4 * m + gp] = np.asarray(W2)
    ws32 = np.ascontiguousarray(ws32.reshape(128, 256))
    b2t = np.full((128, 1), np.asarray(b2, np.float32)[0], np.float32)
    return {
        "w1a": w1a,
        "w1brep": w1brep,
        "b1t": b1t,
        "ws32": ws32,
        "b2t": b2t,
    }


TRACE = False
LAST_RESULTS = None


@functools.lru_cache(maxsize=1)
def _built_nc():
    return _build(T)


def kernel(**inputs):
    from concourse.bass_utils import run_bass_kernel_spmd

    h = np.asarray(inputs["h"])
    # (B, T, N, D) -> (B, D, T*N) so one DMA per core loads all hT with 8KB runs
    ht = np.ascontiguousarray(
        np.transpose(h, (0, 3, 1, 2)).reshape(B, D, -1)
    ).astype(HID_NP)
    # I pre-permuted to the kernel's row order and laid out (N, T*N) per core
    ifull = np.asarray(inputs["I_full"])
    ip = np.ascontiguousarray(
        np.transpose(ifull[:, :, PERM, :], (0, 2, 1, 3)).reshape(B, N, -1)
    ).astype(HID_NP)
    aux = make_aux_inputs(
        inputs["W1"], inputs["b1"], inputs["W2"], inputs["b2"]
    )

    nc = _built_nc()
    in_maps = [{"ht": ht[c], "ifull": ip[c], **aux} for c in range(NCORES)]
    res = run_bass_kernel_spmd(
        nc, in_maps, core_ids=list(range(NCORES)), trace=TRACE
    )
    global LAST_RESULTS
    LAST_RESULTS = res

    def unshard(name):
        raw = np.stack([res.results[c][name] for c in range(NCORES)])
        raw = raw.reshape(B, N, T, N).transpose(0, 2, 1, 3).astype(np.float32)
        out = np.empty_like(raw)
        out[:, :, PERM, :] = raw
        return np.ascontiguousarray(out)

    return unshard("isp"), unshard("m")


# revision 20
# speedup vs baseline: 1.3562x; 1.0805x over previous
"""Trainium2 Bass kernel for the EdgeMask problem.

Computes, for h (B,T,N,d), I_full (B,T,N,N), MLP params W1 (2d,hid) b1 (hid,)
W2 (hid,) b2 (1,):
    li = h @ W1[:d]; lj = h @ W1[d:]
    hid = relu(li[:,:,:,None,:] + lj[:,:,None,:,:] + b1)
    M = sigmoid(hid @ W2 + b2);  I_sparse = I_full * M
Returns (I_sparse, M).

Sharding: data-parallel over B across 8 NeuronCores (B=8), no collectives.

Per-core layout (per t slice, N=128 nodes, d=128, K=32 hidden), processed in
slice PAIRS so fixed per-slice ops amortize:
  - hT = h[t].T (d on partitions), fp16.
  - R = ljT replicated 4x via ONE matmul with a column-replicated lhsT
    (w1brep[d, 32gp+k] = W1b[d,k]); S = li stack via 4 col-block matmuls.
    Partition stacking p = 32*gp + k. Both slices of a pair share one PSUM
    tile; one 256-col R exit (pure copy) + one 64-col S exit (+b1).
  - Pointwise hot loop (N^2*K): group g covers rows i in {g+32gp}:
      hid_g[32gp+k, j] = relu(R[:,j] + S[:,g])   (tensor_scalar add,max)
    split across DVE / GPSIMD / ACT.
  - Reduce over k on PE: per g (= 8q+m) one matmul with a 32-col lhsT
    that is zero except a 4-col stripe (ws32[32gp+k, 32m+4m'+gp] = W2[k]
    at stripe m'=m), 8 matmuls accumulating into PSUM block [32q, 32q+32):
      mp[32q+4m+gp, j] = logits[g+32gp, j]   -- ROW-PERMUTED dense logits.
    Rows are unpermuted on the host (I_full pre-permuted to match).
  - Two slices' logits share one PSUM tile -> one 256-col sigmoid on ACT,
    one 256-col mask multiply on DVE. M and I_sparse go to separate DRAM
    tensors, stored with 4-slice-batched DMAs.
"""

import functools

import numpy as np

import concourse.bass as bass
import concourse.mybir as mybir
import concourse.tile as tile
from concourse import bacc

F32 = mybir.dt.float32
F16 = mybir.dt.float16

B = 8
T = 32
N = 128
D = 128
K = 32  # hidden
NCORES = 8

AFT = mybir.ActivationFunctionType
ALU = mybir.AluOpType

HID_DT = F16
HID_NP = np.float16

# pointwise engine split: counts per 32 groups
ACT_G = 5
POOL_G = 7
# fixed-op engine assignment
R_ENG = "act"    # paired R exit: pure copy PSUM->SBUF fp16, 256 cols (GPSIMD cannot touch PSUM)
S_ENG = "act"    # paired S exit: +b1, 64 cols, fp32 (GPSIMD cannot touch PSUM)
MASK_ENG = "dve"
IO_BATCH = 2
HID_BUFS = 24
RS_BUFS = 3
IO_BUFS = 3
OUT_BUFS = 3
LILJ_BUFS = 3
MP_BUFS = 2


def _pw_seq():
    """Interleaved engine sequence for the 32 pointwise groups."""
    counts = {"act": ACT_G, "pool": POOL_G, "dve": K - ACT_G - POOL_G}
    seq = []
    acc = {e: 0.0 for e in counts}
    for _ in range(K):
        for e in counts:
            acc[e] += counts[e] / K
        pick = max(acc, key=lambda e: acc[e])
        acc[pick] -= 1.0
        seq.append(pick)
    return seq


def _reduce_order():
    """Groups ordered by estimated hid completion time (per slice).

    The reduce consumes groups in this order; row placement is undone by the
    host-side PERM, so the mapping is free. DVE-produced groups finish on a
    ~94ns cadence, Pool ~273ns, ACT ~292ns after its R/S exits.
    """
    seq = _pw_seq()
    cost = {"dve": 94.0, "pool": 273.0, "act": 292.0}
    offset = {"dve": 0.0, "pool": 0.0, "act": 636.0}
    idx = {"dve": 0, "pool": 0, "act": 0}
    done = []
    for g in range(K):
        e = seq[g]
        idx[e] += 1
        done.append((offset[e] + cost[e] * idx[e], g))
    return [g for _, g in sorted(done)]


def _eng_ts(nc, eng, dst, src, scalar1, scalar2, op0, op1=None):
    ns = {"pool": nc.gpsimd, "act": nc.scalar, "dve": nc.vector}[eng]
    if op1 is None:
        ns.tensor_scalar(dst, src, scalar1, scalar2, op0)
    else:
        ns.tensor_scalar(dst, src, scalar1, scalar2, op0, op1)


def _eng_copy(nc, eng, dst, src):
    if eng == "pool":
        nc.gpsimd.tensor_copy(dst, src)
    elif eng == "act":
        nc.scalar.copy(dst, src)
    else:
        nc.vector.tensor_copy(dst, src)


def _build(t_slices: int = T):
    nc = bacc.Bacc(
        "TRN2", target_bir_lowering=False, debug=False, num_devices=NCORES
    )

    ht_d = nc.dram_tensor("ht", [D, t_slices * N], HID_DT, kind="ExternalInput")
    i_d = nc.dram_tensor("ifull", [N, t_slices * N], HID_DT, kind="ExternalInput")
    # merged fp16 consts + the first pair's hT, loaded in ONE startup DMA:
    # [w1a (K) | w1brep (128) | ws32 (256) | ht[:, 0:2N] (256)]
    aux16_d = nc.dram_tensor(
        "aux16", [D, K + 128 + 256 + 2 * N], HID_DT, kind="ExternalInput"
    )
    aux32_d = nc.dram_tensor("aux32", [128, 2], F32, kind="ExternalInput")

    # row-permuted outputs
    m_d = nc.dram_tensor("m", [N, t_slices * N], HID_DT, kind="ExternalOutput")
    isp_d = nc.dram_tensor("isp", [N, t_slices * N], HID_DT, kind="ExternalOutput")

    pw_seq = _pw_seq()
    red_order = _reduce_order()

    with tile.TileContext(nc) as tc:
        with (
            tc.tile_pool(name="const", bufs=1) as cpool,
            tc.tile_pool(name="rs", bufs=RS_BUFS) as rspool,
            tc.tile_pool(name="hid", bufs=HID_BUFS) as hidpool,
            tc.tile_pool(name="io", bufs=IO_BUFS) as iopool,
            tc.tile_pool(name="outp", bufs=OUT_BUFS) as opool,
            tc.tile_pool(name="psum", bufs=2, space="PSUM") as ppool,
        ):
            # one merged DMA delivers all fp16 params + the first pair's hT
            aux16_sb = cpool.tile([D, K + 128 + 256 + 2 * N], HID_DT)
            nc.sync.dma_start(aux16_sb[:], aux16_d[:])
            w1a_sb = aux16_sb[:, 0:K]
            w1brep_sb = aux16_sb[:, K : K + 128]
            ws32_sb = aux16_sb[:, K + 128 : K + 128 + 256]
            ht01_sb = aux16_sb[:, K + 128 + 256 : K + 128 + 256 + 2 * N]

            htall_sb = cpool.tile([D, t_slices * N], HID_DT)
            # first chunk covers slices 2..5 (pair 0 rides in aux16)
            nc.sync.dma_start(
                htall_sb[:, 2 * N : 6 * N], ht_d[:, 2 * N : 6 * N]
            )
            aux32_sb = cpool.tile([128, 2], F32)
            nc.sync.dma_start(aux32_sb[:], aux32_d[:])
            b1t_sb = aux32_sb[:, 0:1]
            b2t_sb = aux32_sb[:, 1:2]

            i4_cur = None
            i4_next = iopool.tile([128, IO_BATCH * N], HID_DT, tag="i4")
            nc.sync.dma_start(i4_next[:], i_d[:, 0 : IO_BATCH * N])
            m4_sb = None
            isp4_sb = None
            pending = None

            # remaining hT chunks (4 slices each)
            for c0 in range(6, t_slices, 4):
                c1 = min(c0 + 4, t_slices)
                nc.sync.dma_start(
                    htall_sb[:, c0 * N : c1 * N], ht_d[:, c0 * N : c1 * N]
                )

            for t0 in range(0, t_slices, 2):
                slot = t0 % IO_BATCH
                if slot == 0:
                    i4_cur = i4_next
                    if t0 + IO_BATCH < t_slices:
                        i4_next = iopool.tile([128, IO_BATCH * N], HID_DT, tag="i4")
                        nc.sync.dma_start(
                            i4_next[:],
                            i_d[:, (t0 + IO_BATCH) * N : (t0 + 2 * IO_BATCH) * N],
                        )
                    m4_sb = opool.tile([128, IO_BATCH * N], HID_DT, tag="m4")
                    isp4_sb = opool.tile([128, IO_BATCH * N], HID_DT, tag="isp4")

                # ---- lilj for both slices of the pair into one PSUM tile ----
                # layout: [R(t0) | R(t0+1) | S(t0) | S(t0+1)]
                lilj_ps = ppool.tile(
                    [128, 2 * N + 2 * K], F32, tag="lilj", bufs=LILJ_BUFS
                )
                for h in range(2):
                    t = t0 + h
                    if t < 2:
                        ht_sb = ht01_sb[:, h * N : (h + 1) * N]
                    else:
                        ht_sb = htall_sb[:, t * N : (t + 1) * N]
                    nc.tensor.matmul(
                        lilj_ps[:, h * N : (h + 1) * N],
                        w1brep_sb[:],
                        ht_sb,
                        skip_group_check=True,
                    )
                    for gp in range(4):
                        nc.tensor.matmul(
                            lilj_ps[
                                32 * gp : 32 * gp + 32,
                                2 * N + h * K : 2 * N + (h + 1) * K,
                            ],
                            w1a_sb[:],
                            ht_sb[:, 32 * gp : 32 * gp + 32],
                            tile_position=(0, 32 * gp),
                            skip_group_check=True,
                        )

                # paired exits: R (256 cols, pure copy), S (64 cols, +b1)
                r2_sb = rspool.tile([128, 2 * N], HID_DT, tag="r2")
                _eng_copy(nc, R_ENG, r2_sb[:], lilj_ps[:, 0 : 2 * N])
                s2_sb = rspool.tile([128, 2 * K], F32, tag="s2")
                if S_ENG == "act":
                    nc.scalar.activation(
                        s2_sb[:], lilj_ps[:, 2 * N : 2 * N + 2 * K],
                        AFT.Identity, bias=b1t_sb,
                    )
                else:
                    _eng_ts(
                        nc, S_ENG, s2_sb[:], lilj_ps[:, 2 * N : 2 * N + 2 * K],
                        b1t_sb, None, ALU.add,
                    )

                if pending is not None:
                    pending()
                    pending = None

                # per-pair PSUM logits tile
                mp2_ps = ppool.tile([128, 2 * N], F32, tag="mp", bufs=MP_BUFS)

                for h in range(2):
                    r_sb = r2_sb[:, h * N : (h + 1) * N]
                    s_sb = s2_sb[:, h * K : (h + 1) * K]

                    # ---- pointwise: hid_g = relu(R + S[:, g]) ----
                    hbufs = [
                        hidpool.tile([128, 4 * N], HID_DT, tag="hid", name=f"hb{w}")
                        for w in range(8)
                    ]
                    for g in range(K):
                        w, c = divmod(g, 4)
                        dst = hbufs[w][:, c * N : (c + 1) * N]
                        eng = pw_seq[g]
                        if eng == "act":
                            nc.scalar.activation(
                                dst, r_sb, AFT.Relu, bias=s_sb[:, g : g + 1]
                            )
                        else:
                            _eng_ts(
                                nc, eng, dst, r_sb, s_sb[:, g : g + 1], 0.0,
                                ALU.add, ALU.max,
                            )

                    # ---- reduce over k on PE: row-permuted dense logits ----
                    # issue r -> group g=red_order[r]; q,m = divmod(r, 8):
                    #   mp[32q+4m+gp, h*N+j] = logits[g+32gp, j]
                    for r in range(K):
                        g = red_order[r]
                        w, c = divmod(g, 4)
                        q, m = divmod(r, 8)
                        nc.tensor.matmul(
                            mp2_ps[32 * q : 32 * q + 32, h * N : (h + 1) * N],
                            ws32_sb[:, 32 * m : 32 * m + 32],
                            hbufs[w][:, c * N : (c + 1) * N],
                            start=(m == 0),
                            stop=(m == 7),
                            tile_position=(0, 32 * q),
                            skip_group_check=True,
                        )

                # ---- pair epilogue (sigmoid, mask, stores), emitted one
                # pair LATE to avoid head-of-line blocking on in-order queues
                def _epilogue(
                    mp2_ps=mp2_ps, m4_sb=m4_sb, isp4_sb=isp4_sb,
                    i4_cur=i4_cur, slot=slot, t0=t0,
                ):
                    pcol = slot * N
                    nc.scalar.activation(
                        m4_sb[:, pcol : pcol + 2 * N],
                        mp2_ps[:],
                        AFT.Sigmoid,
                        bias=b2t_sb,
                    )
                    mask_ns = nc.gpsimd if MASK_ENG == "pool" else nc.vector
                    mask_ns.tensor_tensor(
                        isp4_sb[:, pcol : pcol + 2 * N],
                        i4_cur[:, pcol : pcol + 2 * N],
                        m4_sb[:, pcol : pcol + 2 * N],
                        ALU.mult,
                    )
                    if slot + 2 == IO_BATCH:
                        base = (t0 + 2 - IO_BATCH) * N
                        nc.sync.dma_start(
                            m_d[:, base : base + IO_BATCH * N], m4_sb[:]
                        )
                        nc.sync.dma_start(
                            isp_d[:, base : base + IO_BATCH * N], isp4_sb[:]
                        )

                pending = _epilogue

            if pending is not None:
                pending()

    nc.compile()
    return nc


# permutation: PSUM/DRAM row p' = 32q+4m+gp holds logits row
# i = red_order[8q+m] + 32*gp
def _perm():
    order = _reduce_order()
    return np.array(
        [order[8 * (p // 32) + (p % 32) // 4] + 32 * (p % 4) for p in range(128)],
        dtype=np.int64,
    )


PERM = _perm()


def make_aux_inputs(W1, b1, W2, b2):
    W1 = np.asarray(W1)
    w1a = W1[:D].astype(HID_NP)
    w1b = W1[D:].astype(HID_NP)  # (D, K)
    # w1brep[d, 32gp+k] = W1b[d, k]
    w1brep = np.tile(w1b, (1, 4)).astype(HID_NP)
    b1t = np.tile(np.asarray(b1, np.float32), 4).reshape(128, 1)
    # ws32[:, 32m:32m+32][32gp+k, 4m+gp] = W2[k]; zero elsewhere
    ws32 = np.zeros((128, 8, 32), HID_NP)
    for m in range(8):
        for gp in range(4):
            ws32[32 * gp : 32 * gp + 32, m, 4 * m + gp] = np.asarray(W2)
    ws32 = ws32.reshape(128, 256)
    b2t = np.full((128, 1), np.asarray(b2, np.float32)[0], np.float32)
    aux32 = np.ascontiguousarray(np.concatenate([b1t, b2t], axis=1))
    return {"w1a16": w1a, "w1brep16": w1brep, "ws3216": ws32, "aux32": aux32}


TRACE = False
LAST_RESULTS = None


@functools.lru_cache(maxsize=1)
def _built_nc():
    return _build(T)


def kernel(**inputs):
    from concourse.bass_utils import run_bass_kernel_spmd

    h = np.asarray(inputs["h"])
    # (B, T, N, D) -> (B, D, T*N) so one DMA per core loads all hT with 8KB runs
    ht = np.ascontiguousarray(
        np.transpose(h, (0, 3, 1, 2)).reshape(B, D, -1)
    ).astype(HID_NP)
    # I pre-permuted to the kernel's row order and laid out (N, T*N) per core
    perm = _perm()
    ifull = np.asarray(inputs["I_full"])
    ip = np.ascontiguousarray(
        np.transpose(ifull[:, :, perm, :], (0, 2, 1, 3)).reshape(B, N, -1)
    ).astype(HID_NP)
    aux = make_aux_inputs(
        inputs["W1"], inputs["b1"], inputs["W2"], inputs["b2"]
    )

    nc = _built_nc()
    in_maps = []
    for c in range(NCORES):
        aux16 = np.ascontiguousarray(
            np.concatenate(
                [aux["w1a16"], aux["w1brep16"], aux["ws3216"], ht[c, :, : 2 * N]],
                axis=1,
            )
        )
        in_maps.append(
            {"ht": ht[c], "ifull": ip[c], "aux16": aux16, "aux32": aux["aux32"]}
        )
    res = run_bass_kernel_spmd(
        nc, in_maps, core_ids=list(range(NCORES)), trace=TRACE
    )
    global LAST_RESULTS
    LAST_RESULTS = res

    def unshard(name):
        raw = np.stack([res.results[c][name] for c in range(NCORES)])
        raw = raw.reshape(B, N, T, N).transpose(0, 2, 1, 3).astype(np.float32)
        out = np.empty_like(raw)
        out[:, :, perm, :] = raw
        return np.ascontiguousarray(out)

    return unshard("isp"), unshard("m")
